# revision 10
# baseline (speedup 1.0000x reference)
"""Trainium2 Bass kernel for nn_DRGCNLayer (gnn_message_passing).

Strategy (8 NeuronCores, SPMD, no collectives):
  - Host: sort edges by dst, split into 8 contiguous dst ranges with ~equal
    edge counts -> each core owns a disjoint set of destination nodes.
  - Host packs each core's edges into "blocks" of <=128 dst nodes and
    exactly CH*128 edge slots (padded); dst-local slot ids drive an
    is_equal selection matrix so segment softmax/sum happens in PSUM via
    accumulating matmuls (no indirect scatter, no max pass - scores are
    shift-invariant).
  - q.bk term dropped entirely: softmax is invariant to per-dst constants.
  - time mask sigmoid((ts[dst]-et)/|c|) precomputed on host (tm).
  - rel_table[et] sent feature-major; tm*relp[et]+b1 sent edge-major so h
    is computed edge-major and z = W2.h is a DVE free-dim accumulate.
  - dyn sigmoid via tanh (exp_and_others act table; zero table reloads):
    sigmoid(z) = 0.5 + 0.5*tanh(z/2); the 2x fold absorbed into q scale
    (0.125) and Wv scale (0.5).
  - DVE ops use scalar_tensor_tensor (4x/2x DVE perf modes) where
    possible; k/v evacuated to SBUF bf16 by the scalar engine; v
    head-minor so the attention broadcast multiply keeps a packed
    innermost dim; selection matrices built on gpsimd.
"""
import os
os.environ.setdefault("JAX_PLATFORMS", "axon,cpu")
import numpy as np
import ml_dtypes

BF = ml_dtypes.bfloat16
F32 = np.float32

N = 50000
E = 800000
H = 128
NR = 64
NH = 8
HD = 16
P = 128
NCORES = 8
NMAXN = 6656          # per-core node slots (13 * 512)
CH = 16               # chunks (of 128 edges) per block
EPB = CH * P          # 2048 edge slots per block
SPB = 4               # supersteps per block (4 chunks of 128 each)
WSS = 512             # superstep width in edges
PAD_SLOT = 999.0      # dst-local for pad edges

# consts_bf16 column map
IOTA128 = 0
ONES128 = 128
WQ = 256
W1S = 384
WK = 512
WVDM = 640
W2BC = 768
IDENT = 896
NCB = 1024
# consts_f32 column map
BQ125 = 0
BVDM = 1              # [128, 1:129] bv broadcast, head-minor
NCF = 129


def _build(nblk, invc, b2val, nsup0=NMAXN // WSS, n_nodes=N):
    import concourse.bass as bass
    import concourse.bacc as bacc
    import concourse.mybir as mybir
    import concourse.tile as tile

    f32 = mybir.dt.float32
    bf16 = mybir.dt.bfloat16
    i16 = mybir.dt.int16
    AF = mybir.ActivationFunctionType
    OP = mybir.AluOpType
    nmaxn = nsup0 * WSS

    nc = bacc.Bacc("TRN2", target_bir_lowering=False, debug=False)

    xb2 = nc.declare_dram_parameter("xb2", [n_nodes // 2, 2 * H], bf16, isOutput=False)
    xTb = nc.declare_dram_parameter("xTb", [P, nmaxn], bf16, isOutput=False)
    si16 = nc.declare_dram_parameter("si16", [P, nblk * P], i16, isOutput=False)
    qi16 = nc.declare_dram_parameter("qi16", [P, nblk * P], i16, isOutput=False)
    metaf = nc.declare_dram_parameter("metaf", [P, nblk * 32], f32, isOutput=False)
    tmb = nc.declare_dram_parameter("tmb", [P, nblk * CH], bf16, isOutput=False)
    relrtm = nc.declare_dram_parameter("relrtm", [P, nblk * 2 * EPB], bf16,
                                       isOutput=False)
    cb = nc.declare_dram_parameter("cb", [P, NCB], bf16, isOutput=False)
    cf = nc.declare_dram_parameter("cf", [P, NCF], f32, isOutput=False)
    qcat = nc.dram_tensor("qcat", [nmaxn, H], bf16)
    outb = nc.declare_dram_parameter("outb", [nblk * P, H], f32, isOutput=True)

    with tile.TileContext(nc) as tc:
        with (
            tc.tile_pool(name="cst", bufs=1) as cst,
            tc.tile_pool(name="sb", bufs=2) as sb,
            tc.tile_pool(name="ps", bufs=1, space="PSUM") as ps,
        ):
            cb_t = cst.tile([P, NCB], bf16)
            nc.sync.dma_start(out=cb_t[:], in_=cb[:])
            cf_t = cst.tile([P, NCF], f32)
            nc.sync.dma_start(out=cf_t[:], in_=cf[:])

            iota128 = cb_t[:, IOTA128:IOTA128 + 128]
            ones128 = cb_t[:, ONES128:ONES128 + 128]
            wq_v = cb_t[:, WQ:WQ + 128]
            w1s_v = cb_t[:, W1S:W1S + 128]
            wk_v = cb_t[:, WK:WK + 128]
            wv_v = cb_t[:, WVDM:WVDM + 128]
            w2bc_v = cb_t[:, W2BC:W2BC + 128]
            ident_v = cb_t[:, IDENT:IDENT + 128]
            bq_v = cf_t[:, BQ125:BQ125 + 1]
            bvdm_v = cf_t[:, BVDM:BVDM + 128]

            # ---------------- phase 0: qcat table (q = 0.125*(x@Wq+bq)) -----
            for s0 in range(nsup0):
                xt = sb.tile([P, WSS], bf16, tag="p0xt")
                nc.sync.dma_start(out=xt[:], in_=xTb[:, s0 * WSS:(s0 + 1) * WSS])
                qps = ps.tile([P, WSS], f32, tag="hps")
                nc.tensor.matmul(qps[:], lhsT=wq_v, rhs=xt[:], start=True, stop=True)
                qs = sb.tile([P, WSS], bf16, tag="p0qs")
                nc.scalar.activation(out=qs[:], in_=qps[:], func=AF.Identity,
                                     scale=0.125, bias=bq_v)
                qT = ps.tile([P, WSS], bf16, tag="srcT")
                for j in range(4):
                    nc.tensor.transpose(out=qT[:, j * P:(j + 1) * P],
                                        in_=qs[:, j * P:(j + 1) * P],
                                        identity=ident_v)
                qc = sb.tile([P, WSS], bf16, tag="p0qc")
                nc.vector.tensor_copy(out=qc[:], in_=qT[:])
                for j in range(4):
                    nc.sync.dma_start(
                        out=qcat[s0 * WSS + j * P: s0 * WSS + (j + 1) * P, :],
                        in_=qc[:, j * P:(j + 1) * P])

            tc.strict_bb_all_engine_barrier()

            # ---------------- phase 1: edges ----------------
            def load_block(b):
                """Issue all DMAs + gathers for block b; returns tile dict."""
                si_t = sb.tile([P, P], i16, tag="si16")
                nc.sync.dma_start(out=si_t[:], in_=si16[:, b * P:(b + 1) * P])
                qi_t = sb.tile([P, P], i16, tag="qi16")
                nc.sync.dma_start(out=qi_t[:], in_=qi16[:, b * P:(b + 1) * P])
                mf = sb.tile([P, 32], f32, tag="metaf")
                nc.sync.dma_start(out=mf[:], in_=metaf[:, b * 32:(b + 1) * 32])
                tmt = sb.tile([P, CH], bf16, tag="tmb")
                nc.sync.dma_start(out=tmt[:], in_=tmb[:, b * CH:(b + 1) * CH])
                rr = sb.tile([P, 2 * EPB], bf16, tag="rr")
                nc.sync.dma_start(
                    out=rr[:], in_=relrtm[:, b * 2 * EPB:(b + 1) * 2 * EPB])
                src2 = sb.tile([P, CH, 2 * H], bf16, tag="src2")
                nc.gpsimd.dma_gather(src2[:], xb2[:], si_t[:], EPB, EPB,
                                     2 * H, single_packet=False)
                qblk = sb.tile([P, CH, H], bf16, tag="qblk")
                nc.gpsimd.dma_gather(qblk[:], qcat[:], qi_t[:], EPB, EPB,
                                     H, single_packet=False)
                return {"mf": mf, "tmt": tmt, "rr": rr, "src2": src2,
                        "qblk": qblk}

            cur = nxt = acc = None
            for sup in range(nblk * SPB):
                b, q = divmod(sup, SPB)
                if sup == 0:
                    cur = load_block(0)
                if q == 0:
                    if b > 0:
                        cur = nxt
                    if b + 1 < nblk:
                        nxt = load_block(b + 1)
                    acc = ps.tile([P, 136], f32, tag="acc")
                mf, tmt, rr = cur["mf"], cur["tmt"], cur["rr"]
                src2, qblk = cur["src2"], cur["qblk"]

                sel4 = mf[:, q * 4:q * 4 + 4]
                dl4 = mf[:, 16 + q * 4:16 + q * 4 + 4]
                tm4 = tmt[:, q * 4:q * 4 + 4]
                lo = src2[:, q * 4:q * 4 + 4, 0:H]
                hi = src2[:, q * 4:q * 4 + 4, H:2 * H]

                # pair-select: src_g = lo + sel*(hi-lo)
                dpr = sb.tile([P, WSS], bf16, tag="dpr")
                nc.vector.tensor_tensor(
                    out=dpr[:].rearrange("p (c f) -> p c f", c=4),
                    in0=hi, in1=lo, op=OP.subtract)
                src_g = sb.tile([P, WSS], bf16, tag="srcg")
                for j in range(4):
                    nc.vector.scalar_tensor_tensor(
                        out=src_g[:, j * P:(j + 1) * P],
                        in0=dpr[:, j * P:(j + 1) * P],
                        scalar=sel4[:, j:j + 1],
                        in1=lo[:, j, :],
                        op0=OP.mult, op1=OP.add)

                # src transpose -> feature-major
                srcT = ps.tile([P, WSS], bf16, tag="srcT")
                for j in range(4):
                    nc.tensor.transpose(out=srcT[:, j * P:(j + 1) * P],
                                        in_=src_g[:, j * P:(j + 1) * P],
                                        identity=ident_v)
                src_fm = sb.tile([P, WSS], bf16, tag="srcfm")
                nc.scalar.activation(out=src_fm[:], in_=srcT[:], func=AF.Copy)

                rel_sl = rr[:, q * WSS:(q + 1) * WSS]

                # m0 = rel[et] * src (feature-major)
                m0 = sb.tile([P, WSS], bf16, tag="m0")
                nc.vector.scalar_tensor_tensor(
                    out=m0[:], in0=rel_sl, scalar=1.0, in1=src_fm[:],
                    op0=OP.mult, op1=OP.mult)

                # h = relu(W1s.T src + (tm*relp[et]+b1))  (edge-major)
                hps = ps.tile([P, WSS], f32, tag="hps")
                for j in range(4):
                    rtm_sl = rr[:, EPB + (q * 4 + j) * H:EPB + (q * 4 + j + 1) * H]
                    nc.tensor.matmul(hps[:, j * P:(j + 1) * P],
                                     lhsT=src_fm[:, j * P:(j + 1) * P], rhs=w1s_v,
                                     start=True, stop=False)
                    nc.tensor.matmul(hps[:, j * P:(j + 1) * P],
                                     lhsT=ident_v, rhs=rtm_sl,
                                     start=False, stop=True)
                h_sb = sb.tile([P, WSS], bf16, tag="hsb")
                nc.scalar.activation(out=h_sb[:], in_=hps[:], func=AF.Relu)

                # z = W2.h per edge (free-dim accumulate), s' = (1+tanh)*tm
                zsc = sb.tile([P, WSS], bf16, tag="zsc")
                z_col = sb.tile([P, 4], f32, tag="zcol")
                for j in range(4):
                    nc.vector.scalar_tensor_tensor(
                        out=zsc[:, j * P:(j + 1) * P],
                        in0=h_sb[:, j * P:(j + 1) * P],
                        scalar=1.0, in1=w2bc_v,
                        op0=OP.mult, op1=OP.mult,
                        accum_out=z_col[:, j:j + 1])
                t4 = sb.tile([P, 4], bf16, tag="t4")
                nc.scalar.activation(out=t4[:], in_=z_col[:], func=AF.Tanh,
                                     scale=0.5, bias=0.5 * float(b2val))
                s_col = sb.tile([P, 4], f32, tag="scol")
                nc.vector.scalar_tensor_tensor(
                    out=s_col[:], in0=t4[:], scalar=1.0, in1=tm4,
                    op0=OP.add, op1=OP.mult)

                # k, v (f32 PSUM -> bf16 SBUF; v head-minor via permuted Wv)
                kps = ps.tile([P, WSS], f32, tag="kps")
                vps = ps.tile([P, WSS], f32, tag="vps")
                for j in range(4):
                    nc.tensor.matmul(kps[:, j * P:(j + 1) * P],
                                     lhsT=m0[:, j * P:(j + 1) * P], rhs=wk_v,
                                     start=True, stop=True)
                    nc.tensor.matmul(vps[:, j * P:(j + 1) * P],
                                     lhsT=m0[:, j * P:(j + 1) * P], rhs=wv_v,
                                     start=True, stop=True)
                k_sb = sb.tile([P, WSS], bf16, tag="ksb")
                nc.scalar.activation(out=k_sb[:], in_=kps[:], func=AF.Copy)
                v_sb = sb.tile([P, WSS], bf16, tag="vsb")
                nc.scalar.activation(out=v_sb[:], in_=vps[:], func=AF.Copy)

                # scores
                qk = sb.tile([P, WSS], bf16, tag="qk")
                nc.vector.scalar_tensor_tensor(
                    out=qk[:].rearrange("p (c f) -> p c f", c=4),
                    in0=k_sb[:].rearrange("p (c f) -> p c f", c=4),
                    scalar=1.0,
                    in1=qblk[:, q * 4:q * 4 + 4, :],
                    op0=OP.mult, op1=OP.mult)
                qkr = sb.tile([P, 32], f32, tag="qkr")
                nc.vector.tensor_reduce(
                    out=qkr[:].rearrange("p (c h) -> p c h", c=4),
                    in_=qk[:].rearrange("p (c h d) -> p c h d", c=4, h=NH),
                    axis=mybir.AxisListType.X, op=OP.add)
                scores = sb.tile([P, 32], f32, tag="scores")
                nc.vector.tensor_tensor(
                    out=scores[:].rearrange("p (c h) -> p c h", c=4),
                    in0=qkr[:].rearrange("p (c h) -> p c h", c=4),
                    in1=s_col[:].unsqueeze(-1).to_broadcast([P, 4, NH]),
                    op=OP.mult)

                # payload [e | ep*v] per chunk; e written straight into pay
                pay = sb.tile([P, 4 * 136], bf16, tag="pay")
                payv = pay[:].rearrange("p (c f) -> p c f", c=4)
                nc.scalar.activation(
                    out=payv[:, :, 0:8],
                    in_=scores[:].rearrange("p (c h) -> p c h", c=4),
                    func=AF.Exp)
                ep = sb.tile([P, 32], bf16, tag="ep")
                nc.vector.tensor_tensor(
                    out=ep[:].rearrange("p (c h) -> p c h", c=4),
                    in0=payv[:, :, 0:8],
                    in1=s_col[:].unsqueeze(-1).to_broadcast([P, 4, NH]),
                    op=OP.mult)
                epv = ep[:].rearrange("p (c h) -> p c h", c=4)
                for j in range(4):
                    nc.vector.scalar_tensor_tensor(
                        out=payv[:, j, 8:136].rearrange("p (d h) -> p d h", d=HD),
                        in0=v_sb[:, j * P:(j + 1) * P]
                            .rearrange("p (d h) -> p d h", d=HD),
                        scalar=1.0,
                        in1=epv[:, j, :].unsqueeze(1).to_broadcast([P, HD, NH]),
                        op0=OP.mult, op1=OP.mult)

                # selection matrices + accumulate into block PSUM
                a_em = sb.tile([P, WSS], bf16, tag="aem")
                for j in range(4):
                    nc.vector.scalar_tensor_tensor(
                        out=a_em[:, j * P:(j + 1) * P],
                        in0=iota128, scalar=dl4[:, j:j + 1], in1=ones128,
                        op0=OP.is_equal, op1=OP.mult)
                for j in range(4):
                    ch = q * 4 + j
                    nc.tensor.matmul(
                        acc[:],
                        lhsT=a_em[:, j * P:(j + 1) * P],
                        rhs=payv[:, j, :],
                        start=(ch == 0), stop=(ch == CH - 1))

                if q == SPB - 1:
                    # finalize block: out = vsum/denom + bv (head-minor)
                    recip = sb.tile([P, 8], f32, tag="recip")
                    nc.vector.reciprocal(out=recip[:], in_=acc[:, 0:8])
                    t1 = sb.tile([P, H], f32, tag="t1")
                    nc.vector.scalar_tensor_tensor(
                        out=t1[:].rearrange("p (d h) -> p d h", d=HD),
                        in0=acc[:, 8:136].rearrange("p (d h) -> p d h", d=HD),
                        scalar=1.0,
                        in1=recip[:].unsqueeze(1).to_broadcast([P, HD, NH]),
                        op0=OP.mult, op1=OP.mult)
                    out_sb = sb.tile([P, H], f32, tag="outsb")
                    nc.vector.tensor_tensor(out=out_sb[:], in0=t1[:], in1=bvdm_v,
                                            op=OP.add)
                    nc.sync.dma_start(out=outb[b * P:(b + 1) * P, :], in_=out_sb[:])

    nc.compile()
    return nc


def _host_prep(x, timestamps, src, dst, edge_type, edge_time, rel_table,
               Wq, bq, Wk, bk, Wv, bv, W1, b1, W2, b2, time_coeff,
               n_nodes=N, n_edges=E, nmaxn=NMAXN):
    """Returns (in_maps, nblk, invc, b2val, assembly) for the SPMD run."""
    N_, E_ = n_nodes, n_edges
    x = np.asarray(x, F32)
    timestamps = np.asarray(timestamps, F32)
    src = np.asarray(src).astype(np.int64)
    dst = np.asarray(dst).astype(np.int64)
    edge_type = np.asarray(edge_type).astype(np.int64)
    edge_time = np.asarray(edge_time, F32)

    invc = 1.0 / (abs(float(np.asarray(time_coeff))) + 1e-9)
    b2val = float(np.asarray(b2).reshape(-1)[0])

    order = np.argsort(dst, kind="stable")
    dst_s = dst[order]
    src_s = src[order]
    et_s = edge_type[order]
    etime_s = edge_time[order]
    # host-precomputed time mask (exact f32 sigmoid)
    dlt = (timestamps[dst_s] - etime_s) * invc
    tm_s = (1.0 / (1.0 + np.exp(-dlt))).astype(F32)
    counts = np.bincount(dst_s, minlength=N_)
    cum = np.concatenate([[0], np.cumsum(counts)])  # cum[n] = edges before node n

    nb = [0]
    for c in range(1, NCORES):
        nb.append(int(np.searchsorted(cum, E_ * c // NCORES)))
    nb.append(N_)

    # per-core blocks
    cores = []
    for c in range(NCORES):
        n0, n1 = nb[c], nb[c + 1]
        assert n1 - n0 <= nmaxn, (n0, n1)
        blocks = []
        n = n0
        while n < n1:
            bn = []
            edges = 0
            while n < n1 and len(bn) < P:
                cn = int(counts[n])
                if cn == 0:
                    n += 1
                    continue
                if edges + cn > EPB:
                    break
                bn.append(n)
                edges += cn
                n += 1
            if bn:
                blocks.append((bn, int(cum[bn[0]]), int(cum[bn[-1] + 1])))
        cores.append(blocks)
    nblk = max(len(b) for b in cores)

    def wrap16(flat):
        base = flat.reshape(P, 16).T.astype(np.int16)
        return np.tile(base, (8, 1))

    # head-minor permutation: col d*8+h <- col h*16+d
    hm = (np.arange(H).reshape(NH, HD).T).reshape(-1)  # hm[d*8+h] = h*16+d

    in_maps = []
    assembly = []
    xb2 = np.ascontiguousarray(x.astype(BF).reshape(N_ // 2, 2 * H))
    relt_f = rel_table.astype(F32)
    relp = rel_table.astype(F32) @ W1[H:2 * H].astype(F32) + W1[2 * H].astype(F32)
    b1_f = b1.astype(F32)

    cb = np.zeros((P, NCB), BF)
    cb[:, IOTA128:IOTA128 + 128] = np.arange(P, dtype=F32)[None, :].astype(BF)
    cb[:, ONES128:ONES128 + 128] = np.ones((P, 128), BF)
    cb[:, WQ:WQ + 128] = Wq.astype(BF)
    cb[:, W1S:W1S + 128] = W1[:H].astype(BF)
    cb[:, WK:WK + 128] = Wk.astype(BF)
    cb[:, WVDM:WVDM + 128] = (0.5 * Wv.astype(F32)[:, hm]).astype(BF)
    cb[:, W2BC:W2BC + 128] = np.broadcast_to(
        W2.astype(F32).reshape(1, H), (P, H)).astype(BF)
    cb[:, IDENT:IDENT + 128] = np.eye(P, dtype=F32).astype(BF)
    cf = np.zeros((P, NCF), F32)
    cf[:, BQ125] = 0.125 * bq.astype(F32)
    cf[:, BVDM:BVDM + 128] = np.broadcast_to(bv.astype(F32)[hm], (P, H))

    for c in range(NCORES):
        n0 = nb[c]
        blocks = cores[c]
        ncn = nb[c + 1] - n0
        xtbuf = np.zeros((nmaxn, H), F32)
        xtbuf[:ncn] = x[n0:nb[c + 1]]
        xTb = np.ascontiguousarray(xtbuf.T).astype(BF)

        si16_a = np.zeros((P, nblk, P), np.int16)
        qi16_a = np.zeros((P, nblk, P), np.int16)
        metaf_a = np.zeros((P, nblk, 2, CH), F32)
        metaf_a[:, :, 1, :] = PAD_SLOT
        tmb_a = np.zeros((P, nblk, CH), F32)
        relrtm_a = np.zeros((P, nblk, 2, EPB), F32)
        asmb = []
        for b, (bn, e0, e1) in enumerate(blocks):
            ne = e1 - e0
            bn_arr = np.asarray(bn)
            sl = slice(e0, e1)
            buf_sidx = np.zeros(EPB, np.int64)
            buf_qidx = np.zeros(EPB, np.int64)
            buf_dstl = np.full(EPB, PAD_SLOT, F32)
            buf_tm = np.zeros(EPB, F32)
            buf_et = np.zeros(EPB, np.int64)
            buf_sidx[:ne] = src_s[sl]
            buf_qidx[:ne] = dst_s[sl] - n0
            buf_dstl[:ne] = np.searchsorted(bn_arr, dst_s[sl]).astype(F32)
            buf_tm[:ne] = tm_s[sl]
            buf_et[:ne] = et_s[sl]
            # slot (t*128 + p) -> chunk t, partition p
            si16_a[:, b, :] = wrap16(buf_sidx // 2)
            qi16_a[:, b, :] = wrap16(buf_qidx)
            metaf_a[:, b, 0, :] = (buf_sidx & 1).astype(F32).reshape(CH, P).T
            metaf_a[:, b, 1, :] = buf_dstl.reshape(CH, P).T
            tmb_a[:, b, :] = buf_tm.reshape(CH, P).T
            # feature-major rel[et]: [128, EPB]
            relrtm_a[:, b, 0, :] = relt_f[buf_et].T
            # edge-major tm*relp[et]+b1, packed [p, chunk*H]
            rtm = relp[buf_et] * buf_tm[:, None] + b1_f  # [EPB, H]
            relrtm_a[:, b, 1, :] = (rtm.reshape(CH, P, H)
                                    .transpose(1, 0, 2).reshape(P, EPB))
            asmb.append(bn_arr)
        assembly.append(asmb)
        in_maps.append({
            "xb2": xb2,
            "xTb": xTb,
            "cb": cb,
            "cf": cf,
            "si16": np.ascontiguousarray(si16_a.reshape(P, nblk * P)),
            "qi16": np.ascontiguousarray(qi16_a.reshape(P, nblk * P)),
            "metaf": np.ascontiguousarray(metaf_a.reshape(P, nblk * 32)),
            "tmb": np.ascontiguousarray(tmb_a.reshape(P, nblk * CH)).astype(BF),
            "relrtm": np.ascontiguousarray(
                relrtm_a.reshape(P, nblk * 2 * EPB)).astype(BF),
        })
    return in_maps, nblk, invc, b2val, assembly


def _run(inputs, n_nodes=N, n_edges=E, nmaxn=NMAXN, trace=False):
    from concourse.bass_utils import run_bass_kernel_spmd
    in_maps, nblk, invc, b2val, assembly = _host_prep(
        **inputs, n_nodes=n_nodes, n_edges=n_edges, nmaxn=nmaxn)
    nc = _build(nblk, invc, b2val, nsup0=nmaxn // WSS, n_nodes=n_nodes)
    res = run_bass_kernel_spmd(nc, in_maps, list(range(NCORES)), trace=trace)
    # head-minor -> head-major inverse permutation
    hm = (np.arange(H).reshape(NH, HD).T).reshape(-1)
    out = np.zeros((n_nodes, H), F32)
    for c in range(NCORES):
        ob = res.results[c]["outb"]
        for b, bn_arr in enumerate(assembly[c]):
            out[bn_arr[:, None], hm[None, :]] = ob[b * P:b * P + len(bn_arr)]
    return out, res, nc


def kernel(**inputs):
    out, _res, _nc = _run(inputs)
    return out


# revision 11
# speedup vs baseline: 1.6015x; 1.6015x over previous
"""Trainium2 Bass kernel for nn_DRGCNLayer (gnn_message_passing).

Strategy (8 NeuronCores, SPMD, no collectives):
  - Host: sort edges by dst, split into 8 contiguous dst ranges with ~equal
    edge counts -> each core owns a disjoint set of destination nodes.
  - Host packs each core's edges into "blocks" of <=128 dst nodes and
    exactly CH*128 edge slots (padded); dst-local slot ids drive an
    is_equal selection matrix so segment softmax/sum happens in PSUM via
    accumulating matmuls (no indirect scatter, no max pass - scores are
    shift-invariant).
  - q.bk term dropped entirely: softmax is invariant to per-dst constants.
  - time mask sigmoid((ts[dst]-et)/|c|) precomputed on host (tm).
  - rel_table[et] and tm*relp[et] sent as host-precomputed feature-major
    tiles (no one-hot matmuls on device).
  - dyn sigmoid via tanh (exp_and_others act table; zero table reloads):
    sigmoid(z) = 0.5 + 0.5*tanh(z/2); the 2x fold absorbed into q scale
    (0.125) and Wv scale (0.5).
  - DVE ops picked for perf modes: tensor_scalar (4x) for scalar ops,
    tensor_tensor bf16 (2x) for the rest; k/v evacuated PSUM->SBUF in one
    scalar-engine copy; v head-minor so the attention broadcast multiply
    keeps a packed innermost dim.
"""
import os
os.environ.setdefault("JAX_PLATFORMS", "axon,cpu")
import numpy as np
import ml_dtypes

BF = ml_dtypes.bfloat16
F32 = np.float32

N = 50000
E = 800000
H = 128
NR = 64
NH = 8
HD = 16
P = 128
NCORES = 8
NMAXN = 6656          # per-core node slots (13 * 512)
CH = 16               # chunks (of 128 edges) per block
EPB = CH * P          # 2048 edge slots per block
SPB = 4               # supersteps per block (4 chunks of 128 each)
WSS = 512             # superstep width in edges
PAD_SLOT = 999.0      # dst-local for pad edges

# consts_bf16 column map
IOTA128 = 0
WQ = 128
W1S = 256
WK = 384
WVDM = 512
W2C = 640
IDENT = 641
NCB = 769
# consts_f32 column map
B1 = 0
BQ125 = 1
BVDM = 2              # [128, 2:130] bv broadcast, head-minor
NCF = 130


def _build(nblk, invc, b2val, nsup0=NMAXN // WSS, n_nodes=N):
    import concourse.bass as bass
    import concourse.bacc as bacc
    import concourse.mybir as mybir
    import concourse.tile as tile

    f32 = mybir.dt.float32
    bf16 = mybir.dt.bfloat16
    i16 = mybir.dt.int16
    AF = mybir.ActivationFunctionType
    OP = mybir.AluOpType
    nmaxn = nsup0 * WSS

    nc = bacc.Bacc("TRN2", target_bir_lowering=False, debug=False)

    xb2 = nc.declare_dram_parameter("xb2", [n_nodes // 2, 2 * H], bf16, isOutput=False)
    xTb = nc.declare_dram_parameter("xTb", [P, nmaxn], bf16, isOutput=False)
    si16 = nc.declare_dram_parameter("si16", [P, nblk * P], i16, isOutput=False)
    qi16 = nc.declare_dram_parameter("qi16", [P, nblk * P], i16, isOutput=False)
    metaf = nc.declare_dram_parameter("metaf", [P, nblk * 32], f32, isOutput=False)
    tmb = nc.declare_dram_parameter("tmb", [P, nblk * CH], bf16, isOutput=False)
    relrtm = nc.declare_dram_parameter("relrtm", [P, nblk * 2 * EPB], bf16,
                                       isOutput=False)
    cb = nc.declare_dram_parameter("cb", [P, NCB], bf16, isOutput=False)
    cf = nc.declare_dram_parameter("cf", [P, NCF], f32, isOutput=False)
    qcat = nc.dram_tensor("qcat", [nmaxn, H], bf16)
    outb = nc.declare_dram_parameter("outb", [nblk * P, H], f32, isOutput=True)

    with tile.TileContext(nc) as tc:
        with (
            tc.tile_pool(name="cst", bufs=1) as cst,
            tc.tile_pool(name="sb", bufs=2) as sb,
            tc.tile_pool(name="ps", bufs=1, space="PSUM") as ps,
        ):
            cb_t = cst.tile([P, NCB], bf16)
            nc.sync.dma_start(out=cb_t[:], in_=cb[:])
            cf_t = cst.tile([P, NCF], f32)
            nc.sync.dma_start(out=cf_t[:], in_=cf[:])

            iota128 = cb_t[:, IOTA128:IOTA128 + 128]
            wq_v = cb_t[:, WQ:WQ + 128]
            w1s_v = cb_t[:, W1S:W1S + 128]
            wk_v = cb_t[:, WK:WK + 128]
            wv_v = cb_t[:, WVDM:WVDM + 128]
            w2_v = cb_t[:, W2C:W2C + 1]
            ident_v = cb_t[:, IDENT:IDENT + 128]
            b1_v = cf_t[:, B1:B1 + 1]
            bq_v = cf_t[:, BQ125:BQ125 + 1]
            bvdm_v = cf_t[:, BVDM:BVDM + 128]

            # ---------------- phase 0: qcat table (q = 0.125*(x@Wq+bq)) -----
            for s0 in range(nsup0):
                xt = sb.tile([P, WSS], bf16, tag="p0xt")
                nc.sync.dma_start(out=xt[:], in_=xTb[:, s0 * WSS:(s0 + 1) * WSS])
                qps = ps.tile([P, WSS], f32, tag="hps")
                nc.tensor.matmul(qps[:], lhsT=wq_v, rhs=xt[:], start=True, stop=True)
                qs = sb.tile([P, WSS], bf16, tag="p0qs")
                nc.scalar.activation(out=qs[:], in_=qps[:], func=AF.Identity,
                                     scale=0.125, bias=bq_v)
                qT = ps.tile([P, WSS], bf16, tag="srcT")
                for j in range(4):
                    nc.tensor.transpose(out=qT[:, j * P:(j + 1) * P],
                                        in_=qs[:, j * P:(j + 1) * P],
                                        identity=ident_v)
                qc = sb.tile([P, WSS], bf16, tag="p0qc")
                nc.vector.tensor_copy(out=qc[:], in_=qT[:])
                for j in range(4):
                    nc.sync.dma_start(
                        out=qcat[s0 * WSS + j * P: s0 * WSS + (j + 1) * P, :],
                        in_=qc[:, j * P:(j + 1) * P])

            tc.strict_bb_all_engine_barrier()

            # ---------------- phase 1: edges ----------------
            def load_block(b):
                """Issue all DMAs + gathers for block b; returns tile dict."""
                si_t = sb.tile([P, P], i16, tag="si16")
                nc.sync.dma_start(out=si_t[:], in_=si16[:, b * P:(b + 1) * P])
                qi_t = sb.tile([P, P], i16, tag="qi16")
                nc.sync.dma_start(out=qi_t[:], in_=qi16[:, b * P:(b + 1) * P])
                mf = sb.tile([P, 32], f32, tag="metaf")
                nc.sync.dma_start(out=mf[:], in_=metaf[:, b * 32:(b + 1) * 32])
                tmt = sb.tile([P, CH], bf16, tag="tmb")
                nc.sync.dma_start(out=tmt[:], in_=tmb[:, b * CH:(b + 1) * CH])
                rr = sb.tile([P, 2 * EPB], bf16, tag="rr")
                nc.sync.dma_start(
                    out=rr[:], in_=relrtm[:, b * 2 * EPB:(b + 1) * 2 * EPB])
                src2 = sb.tile([P, CH, 2 * H], bf16, tag="src2")
                nc.gpsimd.dma_gather(src2[:], xb2[:], si_t[:], EPB, EPB,
                                     2 * H, single_packet=False)
                qblk = sb.tile([P, CH, H], bf16, tag="qblk")
                nc.gpsimd.dma_gather(qblk[:], qcat[:], qi_t[:], EPB, EPB,
                                     H, single_packet=False)
                return {"mf": mf, "tmt": tmt, "rr": rr, "src2": src2,
                        "qblk": qblk}

            cur = nxt = acc = None
            for sup in range(nblk * SPB):
                b, q = divmod(sup, SPB)
                if sup == 0:
                    cur = load_block(0)
                if q == 0:
                    if b > 0:
                        cur = nxt
                    if b + 1 < nblk:
                        nxt = load_block(b + 1)
                    acc = ps.tile([P, 136], f32, tag="acc")
                mf, tmt, rr = cur["mf"], cur["tmt"], cur["rr"]
                src2, qblk = cur["src2"], cur["qblk"]

                sel4 = mf[:, q * 4:q * 4 + 4]
                dl4 = mf[:, 16 + q * 4:16 + q * 4 + 4]
                tm4 = tmt[:, q * 4:q * 4 + 4]
                lo = src2[:, q * 4:q * 4 + 4, 0:H]
                hi = src2[:, q * 4:q * 4 + 4, H:2 * H]

                # pair-select: src_g = lo + sel*(hi-lo)
                dpr = sb.tile([P, WSS], bf16, tag="dpr")
                nc.vector.tensor_tensor(
                    out=dpr[:].rearrange("p (c f) -> p c f", c=4),
                    in0=hi, in1=lo, op=OP.subtract)
                dsel = sb.tile([P, WSS], bf16, tag="dsel")
                for j in range(4):
                    nc.vector.tensor_scalar(
                        out=dsel[:, j * P:(j + 1) * P],
                        in0=dpr[:, j * P:(j + 1) * P],
                        scalar1=sel4[:, j:j + 1], scalar2=None,
                        op0=OP.mult)
                src_g = sb.tile([P, WSS], bf16, tag="srcg")
                nc.vector.tensor_tensor(
                    out=src_g[:].rearrange("p (c f) -> p c f", c=4),
                    in0=dsel[:].rearrange("p (c f) -> p c f", c=4),
                    in1=lo, op=OP.add)

                # src transpose -> feature-major
                srcT = ps.tile([P, WSS], bf16, tag="srcT")
                for j in range(4):
                    nc.tensor.transpose(out=srcT[:, j * P:(j + 1) * P],
                                        in_=src_g[:, j * P:(j + 1) * P],
                                        identity=ident_v)
                src_fm = sb.tile([P, WSS], bf16, tag="srcfm")
                nc.scalar.activation(out=src_fm[:], in_=srcT[:], func=AF.Copy)

                rel_sl = rr[:, q * WSS:(q + 1) * WSS]
                rtm_sl = rr[:, EPB + q * WSS:EPB + (q + 1) * WSS]

                # m0 = rel[et] * src (feature-major)
                m0 = sb.tile([P, WSS], bf16, tag="m0")
                nc.vector.tensor_tensor(out=m0[:], in0=rel_sl, in1=src_fm[:],
                                        op=OP.mult)

                # h = relu(W1s.T src + tm*relp[et] + b1)  (feature-major)
                hps = ps.tile([P, WSS], f32, tag="hps")
                nc.tensor.matmul(hps[:], lhsT=w1s_v, rhs=src_fm[:],
                                 start=True, stop=False)
                nc.tensor.matmul(hps[:], lhsT=ident_v, rhs=rtm_sl,
                                 start=False, stop=True)
                h_sb = sb.tile([P, WSS], bf16, tag="hsb")
                nc.scalar.activation(out=h_sb[:], in_=hps[:], func=AF.Relu, bias=b1_v)

                # s' = 2*sigmoid(W2.T h + b2)*tm = (1+tanh((W2.T h + b2)/2))*tm
                dynps = ps.tile([1, WSS], f32, tag="dyn")
                nc.tensor.matmul(dynps[:], lhsT=w2_v, rhs=h_sb[:], start=True, stop=True)
                t_row = sb.tile([1, WSS], bf16, tag="trow")
                nc.scalar.activation(out=t_row[:], in_=dynps[:], func=AF.Tanh,
                                     scale=0.5, bias=0.5 * float(b2val))
                t_col = ps.tile([P, 8], bf16, tag="tcol")
                for j in range(4):
                    nc.tensor.transpose(out=t_col[:, 2 * j:2 * j + 1],
                                        in_=t_row[0:1, j * P:(j + 1) * P],
                                        identity=ident_v[0:1, 0:1])
                s_col = sb.tile([P, 4], f32, tag="scol")
                nc.vector.scalar_tensor_tensor(
                    out=s_col[:],
                    in0=t_col[:].rearrange("p (c two) -> p c two", two=2)[:, :, 0],
                    scalar=1.0, in1=tm4,
                    op0=OP.add, op1=OP.mult)

                # k, v (one f32 PSUM tile -> one bf16 SBUF evac; v head-minor)
                kvps = ps.tile([P, 2 * WSS], f32, tag="kvps")
                for j in range(4):
                    nc.tensor.matmul(kvps[:, j * P:(j + 1) * P],
                                     lhsT=m0[:, j * P:(j + 1) * P], rhs=wk_v,
                                     start=True, stop=True)
                    nc.tensor.matmul(kvps[:, WSS + j * P:WSS + (j + 1) * P],
                                     lhsT=m0[:, j * P:(j + 1) * P], rhs=wv_v,
                                     start=True, stop=True)
                kv_sb = sb.tile([P, 2 * WSS], bf16, tag="kvsb")
                nc.scalar.activation(out=kv_sb[:], in_=kvps[:], func=AF.Copy)
                k_sb = kv_sb[:, 0:WSS]
                v_sb = kv_sb[:, WSS:2 * WSS]

                # scores
                qk = sb.tile([P, WSS], bf16, tag="qk")
                nc.vector.tensor_tensor(
                    out=qk[:].rearrange("p (c f) -> p c f", c=4),
                    in0=k_sb.rearrange("p (c f) -> p c f", c=4),
                    in1=qblk[:, q * 4:q * 4 + 4, :],
                    op=OP.mult)
                qkr = sb.tile([P, 32], f32, tag="qkr")
                nc.vector.tensor_reduce(
                    out=qkr[:].rearrange("p (c h) -> p c h", c=4),
                    in_=qk[:].rearrange("p (c h d) -> p c h d", c=4, h=NH),
                    axis=mybir.AxisListType.X, op=OP.add)
                scores = sb.tile([P, 32], f32, tag="scores")
                nc.vector.tensor_tensor(
                    out=scores[:].rearrange("p (c h) -> p c h", c=4),
                    in0=qkr[:].rearrange("p (c h) -> p c h", c=4),
                    in1=s_col[:].unsqueeze(-1).to_broadcast([P, 4, NH]),
                    op=OP.mult)

                # payload [e | ep*v] per chunk; e written straight into pay
                pay = sb.tile([P, 4 * 136], bf16, tag="pay")
                payv = pay[:].rearrange("p (c f) -> p c f", c=4)
                nc.scalar.activation(
                    out=payv[:, :, 0:8],
                    in_=scores[:].rearrange("p (c h) -> p c h", c=4),
                    func=AF.Exp)
                ep = sb.tile([P, 32], bf16, tag="ep")
                nc.vector.tensor_tensor(
                    out=ep[:].rearrange("p (c h) -> p c h", c=4),
                    in0=payv[:, :, 0:8],
                    in1=s_col[:].unsqueeze(-1).to_broadcast([P, 4, NH]),
                    op=OP.mult)
                nc.vector.tensor_tensor(
                    out=payv[:, :, 8:136].rearrange("p c (d h) -> p c d h", d=HD),
                    in0=v_sb.rearrange("p (c d h) -> p c d h", c=4, d=HD),
                    in1=ep[:].rearrange("p (c h) -> p c h", c=4).unsqueeze(2)
                        .to_broadcast([P, 4, HD, NH]),
                    op=OP.mult)

                # selection matrices + accumulate into block PSUM
                a_em = sb.tile([P, WSS], bf16, tag="aem")
                for j in range(4):
                    nc.vector.tensor_scalar(
                        out=a_em[:, j * P:(j + 1) * P],
                        in0=iota128, scalar1=dl4[:, j:j + 1], scalar2=None,
                        op0=OP.is_equal)
                for j in range(4):
                    ch = q * 4 + j
                    nc.tensor.matmul(
                        acc[:],
                        lhsT=a_em[:, j * P:(j + 1) * P],
                        rhs=payv[:, j, :],
                        start=(ch == 0), stop=(ch == CH - 1))

                if q == SPB - 1:
                    # finalize block: out = vsum/denom + bv (head-minor)
                    recip = sb.tile([P, 8], f32, tag="recip")
                    nc.vector.reciprocal(out=recip[:], in_=acc[:, 0:8])
                    t1 = sb.tile([P, H], f32, tag="t1")
                    nc.vector.scalar_tensor_tensor(
                        out=t1[:].rearrange("p (d h) -> p d h", d=HD),
                        in0=acc[:, 8:136].rearrange("p (d h) -> p d h", d=HD),
                        scalar=1.0,
                        in1=recip[:].unsqueeze(1).to_broadcast([P, HD, NH]),
                        op0=OP.mult, op1=OP.mult)
                    out_sb = sb.tile([P, H], f32, tag="outsb")
                    nc.vector.tensor_tensor(out=out_sb[:], in0=t1[:], in1=bvdm_v,
                                            op=OP.add)
                    nc.sync.dma_start(out=outb[b * P:(b + 1) * P, :], in_=out_sb[:])

    nc.compile()
    return nc


def _host_prep(x, timestamps, src, dst, edge_type, edge_time, rel_table,
               Wq, bq, Wk, bk, Wv, bv, W1, b1, W2, b2, time_coeff,
               n_nodes=N, n_edges=E, nmaxn=NMAXN):
    """Returns (in_maps, nblk, invc, b2val, assembly) for the SPMD run."""
    N_, E_ = n_nodes, n_edges
    x = np.asarray(x, F32)
    timestamps = np.asarray(timestamps, F32)
    src = np.asarray(src).astype(np.int64)
    dst = np.asarray(dst).astype(np.int64)
    edge_type = np.asarray(edge_type).astype(np.int64)
    edge_time = np.asarray(edge_time, F32)

    invc = 1.0 / (abs(float(np.asarray(time_coeff))) + 1e-9)
    b2val = float(np.asarray(b2).reshape(-1)[0])

    order = np.argsort(dst, kind="stable")
    dst_s = dst[order]
    src_s = src[order]
    et_s = edge_type[order]
    etime_s = edge_time[order]
    # host-precomputed time mask (exact f32 sigmoid)
    dlt = (timestamps[dst_s] - etime_s) * invc
    tm_s = (1.0 / (1.0 + np.exp(-dlt))).astype(F32)
    counts = np.bincount(dst_s, minlength=N_)
    cum = np.concatenate([[0], np.cumsum(counts)])  # cum[n] = edges before node n

    nb = [0]
    for c in range(1, NCORES):
        nb.append(int(np.searchsorted(cum, E_ * c // NCORES)))
    nb.append(N_)

    # per-core blocks
    cores = []
    for c in range(NCORES):
        n0, n1 = nb[c], nb[c + 1]
        assert n1 - n0 <= nmaxn, (n0, n1)
        blocks = []
        n = n0
        while n < n1:
            bn = []
            edges = 0
            while n < n1 and len(bn) < P:
                cn = int(counts[n])
                if cn == 0:
                    n += 1
                    continue
                if edges + cn > EPB:
                    break
                bn.append(n)
                edges += cn
                n += 1
            if bn:
                blocks.append((bn, int(cum[bn[0]]), int(cum[bn[-1] + 1])))
        cores.append(blocks)
    nblk = max(len(b) for b in cores)

    def wrap16(flat):
        base = flat.reshape(P, 16).T.astype(np.int16)
        return np.tile(base, (8, 1))

    # head-minor permutation: col d*8+h <- col h*16+d
    hm = (np.arange(H).reshape(NH, HD).T).reshape(-1)  # hm[d*8+h] = h*16+d

    in_maps = []
    assembly = []
    xb2 = np.ascontiguousarray(x.astype(BF).reshape(N_ // 2, 2 * H))
    relt_f = rel_table.astype(F32)
    relp = rel_table.astype(F32) @ W1[H:2 * H].astype(F32) + W1[2 * H].astype(F32)

    cb = np.zeros((P, NCB), BF)
    cb[:, IOTA128:IOTA128 + 128] = np.arange(P, dtype=F32)[None, :].astype(BF)
    cb[:, WQ:WQ + 128] = Wq.astype(BF)
    cb[:, W1S:W1S + 128] = W1[:H].astype(BF)
    cb[:, WK:WK + 128] = Wk.astype(BF)
    cb[:, WVDM:WVDM + 128] = (0.5 * Wv.astype(F32)[:, hm]).astype(BF)
    cb[:, W2C:W2C + 1] = W2.astype(BF)
    cb[:, IDENT:IDENT + 128] = np.eye(P, dtype=F32).astype(BF)
    cf = np.zeros((P, NCF), F32)
    cf[:, B1] = b1.astype(F32)
    cf[:, BQ125] = 0.125 * bq.astype(F32)
    cf[:, BVDM:BVDM + 128] = np.broadcast_to(bv.astype(F32)[hm], (P, H))

    for c in range(NCORES):
        n0 = nb[c]
        blocks = cores[c]
        ncn = nb[c + 1] - n0
        xtbuf = np.zeros((nmaxn, H), F32)
        xtbuf[:ncn] = x[n0:nb[c + 1]]
        xTb = np.ascontiguousarray(xtbuf.T).astype(BF)

        si16_a = np.zeros((P, nblk, P), np.int16)
        qi16_a = np.zeros((P, nblk, P), np.int16)
        metaf_a = np.zeros((P, nblk, 2, CH), F32)
        metaf_a[:, :, 1, :] = PAD_SLOT
        tmb_a = np.zeros((P, nblk, CH), F32)
        relrtm_a = np.zeros((P, nblk, 2, EPB), F32)
        asmb = []
        for b, (bn, e0, e1) in enumerate(blocks):
            ne = e1 - e0
            bn_arr = np.asarray(bn)
            sl = slice(e0, e1)
            buf_sidx = np.zeros(EPB, np.int64)
            buf_qidx = np.zeros(EPB, np.int64)
            buf_dstl = np.full(EPB, PAD_SLOT, F32)
            buf_tm = np.zeros(EPB, F32)
            buf_et = np.zeros(EPB, np.int64)
            buf_sidx[:ne] = src_s[sl]
            buf_qidx[:ne] = dst_s[sl] - n0
            buf_dstl[:ne] = np.searchsorted(bn_arr, dst_s[sl]).astype(F32)
            buf_tm[:ne] = tm_s[sl]
            buf_et[:ne] = et_s[sl]
            # slot (t*128 + p) -> chunk t, partition p
            si16_a[:, b, :] = wrap16(buf_sidx // 2)
            qi16_a[:, b, :] = wrap16(buf_qidx)
            metaf_a[:, b, 0, :] = (buf_sidx & 1).astype(F32).reshape(CH, P).T
            metaf_a[:, b, 1, :] = buf_dstl.reshape(CH, P).T
            tmb_a[:, b, :] = buf_tm.reshape(CH, P).T
            # feature-major rel[et] and tm*relp[et]: [128, EPB]
            relrtm_a[:, b, 0, :] = relt_f[buf_et].T
            relrtm_a[:, b, 1, :] = (relp[buf_et] * buf_tm[:, None]).T
            asmb.append(bn_arr)
        assembly.append(asmb)
        in_maps.append({
            "xb2": xb2,
            "xTb": xTb,
            "cb": cb,
            "cf": cf,
            "si16": np.ascontiguousarray(si16_a.reshape(P, nblk * P)),
            "qi16": np.ascontiguousarray(qi16_a.reshape(P, nblk * P)),
            "metaf": np.ascontiguousarray(metaf_a.reshape(P, nblk * 32)),
            "tmb": np.ascontiguousarray(tmb_a.reshape(P, nblk * CH)).astype(BF),
            "relrtm": np.ascontiguousarray(
                relrtm_a.reshape(P, nblk * 2 * EPB)).astype(BF),
        })
    return in_maps, nblk, invc, b2val, assembly


def _run(inputs, n_nodes=N, n_edges=E, nmaxn=NMAXN, trace=False):
    from concourse.bass_utils import run_bass_kernel_spmd
    in_maps, nblk, invc, b2val, assembly = _host_prep(
        **inputs, n_nodes=n_nodes, n_edges=n_edges, nmaxn=nmaxn)
    nc = _build(nblk, invc, b2val, nsup0=nmaxn // WSS, n_nodes=n_nodes)
    res = run_bass_kernel_spmd(nc, in_maps, list(range(NCORES)), trace=trace)
    # head-minor -> head-major inverse permutation
    hm = (np.arange(H).reshape(NH, HD).T).reshape(-1)
    out = np.zeros((n_nodes, H), F32)
    for c in range(NCORES):
        ob = res.results[c]["outb"]
        for b, bn_arr in enumerate(assembly[c]):
            out[bn_arr[:, None], hm[None, :]] = ob[b * P:b * P + len(bn_arr)]
    return out, res, nc


def kernel(**inputs):
    out, _res, _nc = _run(inputs)
    return out
